# revision 31
# baseline (speedup 1.0000x reference)
"""BiConvLSTM kernel for one TRN2 chip (8 NeuronCores).

Strategy: 8-way model parallelism over contiguous hidden units.
  - Core r owns hidden units 288r..288r+287 and holds the i|f|o|g gate
    rows for those units (1152 rows) of W_ih / W_hh resident in SBUF.
  - The recurrent gate GEMM is column-tiled on the PE array: the four
    gates stream concurrently in the four 32-column groups
    (tile_position (0, 32*gi) via PSUM base partitions 0/32/64/96), so
    the 24 contraction groups take ~24 x 288 cycles instead of 96 x 288.
  - Hidden-state exchange per step is an 8-rank AllGather of each
    core's transposed h slice, padded to a [128, 48] bf16 blob
    (3 slots of 16 seq columns; slot 2 rows 32..127 are junk and are
    cancelled by zero rows in the whh layout).
  - The input projection (x @ W_ih.T + biases) is NOT precomputed up
    front: it is computed in 16-column chunks inside the AllGather gaps
    of earlier steps and injected into the gate PSUM accumulation with
    a K=16 identity matmul (start=True), so no stage DMA / DVE add.
  - Epilogue runs on PSUM column groups: one sigmoid ACT covers i,f,o
    (partitions 0..80), one tanh ACT for g; DVE ops read across
    quadrants (nch<=32 bank-0 routing) so no realign is needed.

Sequence rows 0..7 = fwd samples 0..7, rows 8..15 = bwd samples 0..7.
xproj chunk c holds columns [t=c samples 0..7 | t=15-c samples 0..7];
steps s>=8 reuse chunk 15-s with the half-swapped identity.
"""

import sys

if "/opt/trn_rl_repo" not in sys.path:
    sys.path.append("/opt/trn_rl_repo")

import ml_dtypes
import numpy as np

from concourse import bacc, bass_utils, mybir, tile
from concourse.tile import add_dep_helper

B, T, H, W = 8, 16, 48, 48
HW = H * W              # 2304
NSEQ = 16               # 8 samples x 2 directions
NC = 8                  # cores
S = HW // NC            # 288 hidden units per core
G4 = 4 * S              # 1152 gate rows per core
NG = 24                 # padded contraction groups (8 cores x 3 slots)
KX = 19                 # xproj k-tiles (18 + bias ones-row)
NCH = 8                 # xproj chunks of 16 columns
F32 = mybir.dt.float32
BF16 = mybir.dt.bfloat16

SIG = None  # set in _build
TANH = None


def _build(w0: float, w1: float, cb: float):
    nc = bacc.Bacc("TRN2", target_bir_lowering=False, debug=False, num_devices=NC)

    xT_d = nc.dram_tensor("xT", [128, KX, 128], BF16, kind="ExternalInput")
    wih_d = nc.dram_tensor("wih", [128, KX, G4], BF16, kind="ExternalInput")
    whhF_d = nc.dram_tensor("whhF", [128, NC, 2, G4], BF16, kind="ExternalInput")
    whhQ_d = nc.dram_tensor("whhQ", [32, NC, G4], BF16, kind="ExternalInput")
    eyes_d = nc.dram_tensor("eyes", [16, 32], BF16, kind="ExternalInput")
    out_d = nc.dram_tensor("out", [B, S], F32, kind="ExternalOutput")

    SIG = mybir.ActivationFunctionType.Sigmoid
    TANH = mybir.ActivationFunctionType.Tanh
    ADD = mybir.AluOpType.add
    MULT = mybir.AluOpType.mult
    MAX = mybir.AluOpType.max

    with tile.TileContext(nc) as tc:
        with (
            tc.tile_pool(name="const", bufs=1) as constp,
            tc.tile_pool(name="ew", bufs=2) as ewp,
            tc.tile_pool(name="state", bufs=2) as statep,
            tc.tile_pool(name="blob", bufs=2) as blobp,
            tc.tile_pool(name="hTg", bufs=2) as hTgp,
            tc.tile_pool(name="pg", bufs=3, space="PSUM") as pgp,
            tc.tile_pool(name="pt", bufs=3, space="PSUM") as ptp,
            tc.tile_pool(name="pd", bufs=1, space="PSUM") as pdp,
            tc.tile_pool(name="dram", bufs=2, space="DRAM") as dp,
        ):
            # throwaway collective: pays the ncfw first-call cost while the
            # weight DMAs stream in
            warm_in = dp.tile([128, 48], BF16, tag="warmin")
            warm_out = dp.tile([NC, 128, 48], BF16, addr_space="Shared", tag="warmout")
            nc.gpsimd.collective_compute(
                "AllGather",
                mybir.AluOpType.bypass,
                ins=[warm_in.opt()],
                outs=[warm_out.opt()],
                replica_groups=[list(range(NC))],
            )

            eyes_sb = constp.tile([16, 32], BF16, tag="eyes")
            nc.scalar.dma_start(eyes_sb[:, :], eyes_d[:, :])
            # touch sigmoid/tanh early so the ACT table set loads during the
            # weight DMAs instead of on step 0's critical path
            actwarm = constp.tile([16, 32], F32, tag="actwarm")
            nc.scalar.activation(actwarm[:, :], eyes_sb[:, :], SIG)

            # xT + wih first (xproj chunk 0 gates the whole pipeline); whh is
            # only needed once step 1's gates run (~20us later). One HWDGE
            # queue moves ~130 GB/s, so spread across 4 engine queues.
            xT_sb = constp.tile([128, KX, 128], BF16, tag="xT")
            nc.gpsimd.dma_start(xT_sb[:, 0:10, :], xT_d[:, 0:10, :])
            nc.gpsimd.dma_start(xT_sb[:, 10:KX, :], xT_d[:, 10:KX, :])

            wih_sb = constp.tile([128, KX, G4], BF16, tag="wih")
            wih_engines = [nc.sync, nc.scalar, nc.gpsimd, nc.sync, nc.scalar,
                           nc.gpsimd]
            wih_bounds = [0, 4, 7, 10, 13, 16, KX]
            for piece in range(6):
                k0, k1 = wih_bounds[piece], wih_bounds[piece + 1]
                wih_engines[piece].dma_start(
                    wih_sb[:, k0:k1, :], wih_d[:, k0:k1, :]
                )

            whh_sb = constp.tile([128, NC, 3, G4], BF16, tag="whh")
            # zero rows cancel the junk rows of blob slot 2 (ops starting at
            # partition>0 are limited to 32 partitions -> 3 memsets)
            for p0 in (32, 64, 96):
                nc.vector.memset(whh_sb[p0 : p0 + 32, :, 2, :], 0.0)
            whh_engines = [nc.gpsimd, nc.sync, nc.scalar, nc.gpsimd]
            for piece in range(4):
                r0 = piece * 2
                whh_engines[piece].dma_start(
                    whh_sb[:, r0 : r0 + 2, 0:2, :], whhF_d[:, r0 : r0 + 2, :, :]
                )
            nc.sync.dma_start(whh_sb[0:32, :, 2, :], whhQ_d[:, :, :])

            xp_sb = constp.tile([16, NCH, G4], BF16, tag="xp")

            def xproj_chunk(c):
                px = pgp.tile([128, S], F32, tag="pg", name=f"px{c}")
                for k in range(KX):
                    for gi in range(4):
                        nc.tensor.matmul(
                            px[32 * gi : 32 * gi + 16, :],
                            lhsT=xT_sb[:, k, 16 * c : 16 * c + 16],
                            rhs=wih_sb[:, k, S * gi : S * (gi + 1)],
                            start=(k == 0),
                            stop=(k == KX - 1),
                            tile_position=(0, 32 * gi),
                        )
                for gi in range(4):
                    nc.vector.tensor_copy(
                        xp_sb[0:16, c, S * gi : S * (gi + 1)],
                        px[32 * gi : 32 * gi + 16, :],
                    )

            xproj_chunk(0)
            xproj_chunk(1)

            def inject(pg, s):
                cs = s if s < 8 else 15 - s
                eye_sl = eyes_sb[:, 0:16] if s < 8 else eyes_sb[:, 16:32]
                for gi in range(4):
                    nc.tensor.matmul(
                        pg[32 * gi : 32 * gi + 16, :],
                        lhsT=eye_sl,
                        rhs=xp_sb[0:16, cs, S * gi : S * (gi + 1)],
                        start=True,
                        stop=(s == 0),
                        tile_position=(0, 32 * gi),
                    )

            pg = pgp.tile([128, S], F32, tag="pg", name="pg0")
            inject(pg, 0)

            hTg_prev = None
            c_prev = None
            h15 = None
            for s in range(T):
                if s > 0:
                    for grp in range(NG):
                        r, c = divmod(grp, 3)
                        for gi in range(4):
                            nc.tensor.matmul(
                                pg[32 * gi : 32 * gi + 16, :],
                                lhsT=hTg_prev[:, r, 16 * c : 16 * c + 16],
                                rhs=whh_sb[:, r, c, S * gi : S * (gi + 1)],
                                start=False,
                                stop=(grp == NG - 1),
                                tile_position=(0, 32 * gi),
                            )

                # ---- epilogue: PSUM col groups i@0, f@32, o@64, g@96.
                # Cross-base ACTs realign every gate to partition base 0
                # (walrus only restricts InstTensorTensor to same-base APs).
                if s > 0:
                    sgf = ewp.tile([16, S], F32, tag="sgf")
                    nc.scalar.activation(sgf[:, :], pg[32:48, :], SIG)
                tg = ewp.tile([16, S], F32, tag="tg")
                nc.scalar.activation(tg[:, :], pg[96:112, :], TANH)
                sgi = ewp.tile([16, S], F32, tag="sgi")
                nc.scalar.activation(sgi[:, :], pg[0:16, :], SIG)
                sgo = ewp.tile([16, S], F32, tag="sgo")
                nc.scalar.activation(sgo[:, :], pg[64:80, :], SIG)
                if s > 0:
                    fc = ewp.tile([16, S], F32, tag="fc")
                    nc.vector.tensor_tensor(fc[:, :], sgf[:, :], c_prev[:, :], MULT)
                m1 = ewp.tile([16, S], F32, tag="m1")
                nc.vector.tensor_tensor(m1[:, :], sgi[:, :], tg[:, :], MULT)
                if s == 0:
                    c_new = m1
                else:
                    c_new = statep.tile([16, S], F32, tag="c")
                c_prev = c_new
                tc_ = ewp.tile([16, S], F32, tag="tc")

                CHUNKS = ((0, 0, 128), (1, 128, 256), (2, 256, S))
                if s < T - 1:
                    # two short dummies keep the PE HAM warm through the
                    # scalar/DVE epilogue chain without delaying the
                    # transposes queued behind them
                    dum = pdp.tile([16, S], F32, tag="dum")
                    for _ in range(2):
                        nc.tensor.matmul(
                            dum[:, 0:144], lhsT=tg[:, 0:16], rhs=tg[:, 0:144],
                            start=True, stop=True,
                        )

                    h = statep.tile([16, S], BF16, tag="h")
                    blob = blobp.tile([128, 48], BF16, tag="blob")
                    # slot-2 rows 32.. must be finite: stationary junk is
                    # multiplied by whh zero rows, but NaN*0 would poison PSUM
                    for p0 in (32, 64, 96):
                        nc.gpsimd.memset(blob[p0 : p0 + 32, 32:48], 0.0)
                    # column-chunked tail: c -> tanh(c) -> h -> transpose ->
                    # blob, pipelined across gpsimd/scalar/vector/PE
                    last_tr = None
                    for j, c0, c1 in CHUNKS:
                        if s == 0:
                            nc.scalar.activation(
                                tc_[:, c0:c1], m1[:, c0:c1], TANH
                            )
                        else:
                            nc.gpsimd.tensor_tensor(
                                c_new[:, c0:c1], fc[:, c0:c1], m1[:, c0:c1], ADD
                            )
                            nc.scalar.activation(
                                tc_[:, c0:c1], c_new[:, c0:c1], TANH
                            )
                        nc.vector.tensor_tensor(
                            h[:, c0:c1], sgo[:, c0:c1], tc_[:, c0:c1], MULT
                        )
                        tp = ptp.tile([128, 16], BF16, tag="tp")
                        wdt = c1 - c0
                        tr = nc.tensor.transpose(
                            tp[0:wdt, :], h[:, c0:c1], eyes_sb[:, 0:16]
                        )
                        last_tr = tr
                        nc.vector.tensor_copy(
                            blob[0:wdt, 16 * j : 16 * j + 16], tp[0:wdt, :]
                        )
                    # transport: blob -> DRAM -> AllGather -> hTg SBUF.
                    # cc_in moves per blob slot so the last (smallest) piece
                    # lands right after the final transpose copy
                    cc_in = dp.tile([128, 48], BF16, tag="ccin")
                    nc.sync.dma_start(cc_in[:, 0:16], blob[:, 0:16])
                    nc.scalar.dma_start(cc_in[:, 16:32], blob[:, 16:32])
                    nc.sync.dma_start(cc_in[:, 32:48], blob[:, 32:48])
                    cc_out = dp.tile(
                        [NC, 128, 48], BF16, addr_space="Shared", tag="ccout"
                    )
                    nc.gpsimd.collective_compute(
                        "AllGather",
                        mybir.AluOpType.bypass,
                        ins=[cc_in.opt()],
                        outs=[cc_out.opt()],
                        replica_groups=[list(range(NC))],
                    )
                    hTg_prev = hTgp.tile([128, NC, 48], BF16, tag="hTg")
                    for rr, nr, eng in (
                        (0, 3, nc.sync), (3, 3, nc.scalar), (6, 2, nc.gpsimd)
                    ):
                        eng.dma_start(
                            hTg_prev[:, rr : rr + nr, :],
                            cc_out[rr : rr + nr, :, :].rearrange("r p e -> p r e"),
                        )
                    # gap work: future xproj chunk + next step's inject
                    if s + 2 <= NCH - 1:
                        xproj_chunk(s + 2)
                    pg = pgp.tile([128, S], F32, tag="pg", name=f"pg{s + 1}")
                    inject(pg, s + 1)
                    # dummy f32 matmuls bridge the remaining AllGather gap so
                    # the PE HAM clock-gate stays at 8/8
                    ndum = 9 if s + 2 <= NCH - 1 else 12
                    first_dum2 = None
                    for _ in range(ndum):
                        dmm = nc.tensor.matmul(
                            dum[:, :], lhsT=tg[:, 0:16], rhs=tg[:, 0:S],
                            start=True, stop=True,
                        )
                        if first_dum2 is None:
                            first_dum2 = dmm
                    add_dep_helper(
                        first_dum2.ins, last_tr.ins, False,
                        reason="dummies after transpose",
                    )
                else:
                    if s > 0:
                        nc.gpsimd.tensor_tensor(
                            c_new[:, :], fc[:, :], m1[:, :], ADD
                        )
                    nc.scalar.activation(tc_[:, :], c_new[:, :], TANH)
                    h15 = ewp.tile([16, S], F32, tag="h15")
                    nc.vector.tensor_tensor(h15[:, :], sgo[:, :], tc_[:, :], MULT)

            # ---- output: y[b] = leaky(w0*tanh(h_fwd[b]) + w1*tanh(h_bwd[b]) + cb)
            th = ewp.tile([16, S], F32, tag="th")
            nc.scalar.activation(th[:, :], h15[:, :], TANH)
            thb = ewp.tile([B, S], F32, tag="thb")
            nc.sync.dma_start(thb[:, :], th[8:16, :])
            yb = ewp.tile([B, S], F32, tag="yb")
            nc.vector.tensor_scalar(yb[:, :], thb[:, :], w1, cb, MULT, ADD)
            yc = ewp.tile([B, S], F32, tag="yc")
            nc.vector.scalar_tensor_tensor(yc[:, :], th[0:8, :], w0, yb[:, :], MULT, ADD)
            ye = ewp.tile([B, S], F32, tag="ye")
            nc.vector.scalar_tensor_tensor(ye[:, :], yc[:, :], 0.01, yc[:, :], MULT, MAX)
            nc.sync.dma_start(out_d[:, :], ye[:, :])

    nc.compile()
    return nc


def _prep_inputs(x, W_ih, W_hh, b_ih, b_hh):
    """Build the 8 per-core input maps (SBUF-layout pre-permuted)."""
    X = np.ascontiguousarray(x, dtype=np.float32).reshape(B, T, HW)
    # sequence-row layout (matches the reference's stack+reshape pairing):
    # rows 0-3 fwd samples (0,2,4,6); 4-7 bwd (0,2,4,6);
    # rows 8-11 fwd (1,3,5,7); 12-15 bwd (1,3,5,7)
    SAMP = [0, 2, 4, 6, 0, 2, 4, 6, 1, 3, 5, 7, 1, 3, 5, 7]
    ISFWD = [1, 1, 1, 1, 0, 0, 0, 0, 1, 1, 1, 1, 0, 0, 0, 0]
    cols = np.zeros((HW, 128), dtype=np.float32)
    for ch in range(NCH):
        for j in range(16):
            t = ch if ISFWD[j] else T - 1 - ch
            cols[:, 16 * ch + j] = X[SAMP[j], t, :]
    xTfull = np.zeros((KX * 128, 128), dtype=np.float32)
    xTfull[:HW] = cols
    xTfull[HW] = 1.0  # bias ones-row (k-tile 18, partition 0)
    xT = np.ascontiguousarray(
        xTfull.reshape(KX, 128, 128).transpose(1, 0, 2)
    ).astype(ml_dtypes.bfloat16)

    bias = (b_ih + b_hh).astype(np.float32)
    eye = np.eye(16, dtype=np.float32)
    # steps s>=8 reuse chunk 15-s with fwd/bwd row groups swapped (j ^ 4)
    eyeS = np.zeros((16, 16), dtype=np.float32)
    eyeS[np.arange(16) ^ 4, np.arange(16)] = 1.0
    eyes = np.concatenate([eye, eyeS], axis=1).astype(ml_dtypes.bfloat16)

    in_maps = []
    for q in range(NC):
        units = np.arange(S * q, S * (q + 1))
        # col-group order i, f, o, g  (torch gate blocks i=0, f=1, g=2, o=3)
        rows = np.concatenate([g * HW + units for g in (0, 1, 3, 2)])
        Wi = W_ih[rows].T.astype(np.float32)  # [HW, G4]
        wih_full = np.zeros((KX * 128, G4), dtype=np.float32)
        wih_full[:HW] = Wi
        wih_full[HW] = bias[rows]
        wih = np.ascontiguousarray(
            wih_full.reshape(KX, 128, G4).transpose(1, 0, 2)
        ).astype(ml_dtypes.bfloat16)

        Wt = W_hh[rows].T.astype(np.float32)  # [HW(contraction), G4]
        whhF = np.zeros((128, NC, 2, G4), dtype=np.float32)
        whhQ = np.zeros((32, NC, G4), dtype=np.float32)
        for r in range(NC):
            for c in range(2):
                whhF[:, r, c, :] = Wt[S * r + 128 * c : S * r + 128 * c + 128, :]
            whhQ[:, r, :] = Wt[S * r + 256 : S * r + 288, :]
        in_maps.append(
            {
                "xT": xT,
                "wih": wih,
                "whhF": whhF.astype(ml_dtypes.bfloat16),
                "whhQ": whhQ.astype(ml_dtypes.bfloat16),
                "eyes": eyes,
            }
        )
    return in_maps


def run(x, W_ih, W_hh, b_ih, b_hh, conv_w, conv_b, trace=False, tmpdir=None):
    """Build + run on 8 cores; returns (full_output, BassKernelResults)."""
    w0 = float(np.asarray(conv_w).reshape(2)[0])
    w1 = float(np.asarray(conv_w).reshape(2)[1])
    cb = float(np.asarray(conv_b).reshape(1)[0])
    nc = _build(w0, w1, cb)
    in_maps = _prep_inputs(
        np.asarray(x), np.asarray(W_ih), np.asarray(W_hh),
        np.asarray(b_ih), np.asarray(b_hh),
    )
    res = bass_utils.run_bass_kernel_spmd(
        nc, in_maps, core_ids=list(range(NC)), trace=trace, tmpdir=tmpdir
    )
    y = np.empty((B, HW), dtype=np.float32)
    for q in range(NC):
        y[:, S * q : S * (q + 1)] = res.results[q]["out"]
    return y.reshape(B, 1, H, W).astype(np.float32), res


def kernel(x, W_ih, W_hh, b_ih, b_hh, conv_w, conv_b):
    y, _ = run(x, W_ih, W_hh, b_ih, b_hh, conv_w, conv_b, trace=False)
    return y


# revision 34
# speedup vs baseline: 1.1471x; 1.1471x over previous
"""BiConvLSTM kernel for one TRN2 chip (8 NeuronCores).

Strategy: 8-way model parallelism over contiguous hidden units.
  - Core r owns hidden units 288r..288r+287 and holds the i|f|o|g gate
    rows for those units (1152 rows) of W_ih / W_hh resident in SBUF.
  - The recurrent gate GEMM is column-tiled on the PE array: the four
    gates stream concurrently in the four 32-column groups
    (tile_position (0, 32*gi) via PSUM base partitions 0/32/64/96), so
    the 24 contraction groups take ~24 x 288 cycles instead of 96 x 288.
  - Hidden-state exchange per step is an 8-rank AllGather of each
    core's transposed h slice, padded to a [128, 48] bf16 blob
    (3 slots of 16 seq columns; slot 2 rows 32..127 are junk and are
    cancelled by zero rows in the whh layout).
  - The input projection (x @ W_ih.T + biases) is NOT precomputed up
    front: it is computed in 16-column chunks inside the AllGather gaps
    of earlier steps and injected into the gate PSUM accumulation with
    a K=16 identity matmul (start=True), so no stage DMA / DVE add.
  - Epilogue runs on PSUM column groups: one sigmoid ACT covers i,f,o
    (partitions 0..80), one tanh ACT for g; DVE ops read across
    quadrants (nch<=32 bank-0 routing) so no realign is needed.

Sequence rows 0..7 = fwd samples 0..7, rows 8..15 = bwd samples 0..7.
xproj chunk c holds columns [t=c samples 0..7 | t=15-c samples 0..7];
steps s>=8 reuse chunk 15-s with the half-swapped identity.
"""

import sys

if "/opt/trn_rl_repo" not in sys.path:
    sys.path.append("/opt/trn_rl_repo")

import ml_dtypes
import numpy as np

from concourse import bacc, bass_utils, mybir, tile
from concourse.tile import add_dep_helper

B, T, H, W = 8, 16, 48, 48
HW = H * W              # 2304
NSEQ = 16               # 8 samples x 2 directions
NC = 8                  # cores
S = HW // NC            # 288 hidden units per core
G4 = 4 * S              # 1152 gate rows per core
NG = 24                 # padded contraction groups (8 cores x 3 slots)
KX = 19                 # xproj k-tiles (18 + bias ones-row)
NCH = 8                 # xproj chunks of 16 columns
F32 = mybir.dt.float32
BF16 = mybir.dt.bfloat16

SIG = None  # set in _build
TANH = None


def _build(w0: float, w1: float, cb: float):
    nc = bacc.Bacc("TRN2", target_bir_lowering=False, debug=False, num_devices=NC)

    xT_d = nc.dram_tensor("xT", [128, KX, 128], BF16, kind="ExternalInput")
    wih_d = nc.dram_tensor("wih", [128, KX, G4], BF16, kind="ExternalInput")
    whhF_d = nc.dram_tensor("whhF", [128, NC, 2, G4], BF16, kind="ExternalInput")
    whhQ_d = nc.dram_tensor("whhQ", [32, NC, G4], BF16, kind="ExternalInput")
    eyes_d = nc.dram_tensor("eyes", [16, 32], BF16, kind="ExternalInput")
    out_d = nc.dram_tensor("out", [B, S], F32, kind="ExternalOutput")

    SIG = mybir.ActivationFunctionType.Sigmoid
    TANH = mybir.ActivationFunctionType.Tanh
    ADD = mybir.AluOpType.add
    MULT = mybir.AluOpType.mult
    MAX = mybir.AluOpType.max

    with tile.TileContext(nc) as tc:
        with (
            tc.tile_pool(name="const", bufs=1) as constp,
            tc.tile_pool(name="ew", bufs=2) as ewp,
            tc.tile_pool(name="state", bufs=2) as statep,
            tc.tile_pool(name="blob", bufs=2) as blobp,
            tc.tile_pool(name="hTg", bufs=2) as hTgp,
            tc.tile_pool(name="pg", bufs=3, space="PSUM") as pgp,
            tc.tile_pool(name="pt", bufs=3, space="PSUM") as ptp,
            tc.tile_pool(name="pd", bufs=1, space="PSUM") as pdp,
            tc.tile_pool(name="dram", bufs=2, space="DRAM") as dp,
        ):
            # throwaway collective: pays the ncfw first-call cost while the
            # weight DMAs stream in
            warm_in = dp.tile([128, 48], BF16, tag="warmin")
            warm_out = dp.tile([NC, 128, 48], BF16, addr_space="Shared", tag="warmout")
            nc.gpsimd.collective_compute(
                "AllGather",
                mybir.AluOpType.bypass,
                ins=[warm_in.opt()],
                outs=[warm_out.opt()],
                replica_groups=[list(range(NC))],
            )

            eyes_sb = constp.tile([16, 32], BF16, tag="eyes")
            nc.scalar.dma_start(eyes_sb[:, :], eyes_d[:, :])
            # touch sigmoid/tanh early so the ACT table set loads during the
            # weight DMAs instead of on step 0's critical path
            actwarm = constp.tile([16, 32], F32, tag="actwarm")
            nc.scalar.activation(actwarm[:, :], eyes_sb[:, :], SIG)

            # xT + wih first (xproj chunk 0 gates the whole pipeline); whh is
            # only needed once step 1's gates run (~20us later). One HWDGE
            # queue moves ~130 GB/s, so spread across 4 engine queues.
            xT_sb = constp.tile([128, KX, 128], BF16, tag="xT")
            nc.gpsimd.dma_start(xT_sb[:, 0:10, :], xT_d[:, 0:10, :])
            nc.gpsimd.dma_start(xT_sb[:, 10:KX, :], xT_d[:, 10:KX, :])

            wih_sb = constp.tile([128, KX, G4], BF16, tag="wih")
            wih_engines = [nc.sync, nc.scalar, nc.gpsimd, nc.sync, nc.scalar,
                           nc.gpsimd]
            wih_bounds = [0, 4, 7, 10, 13, 16, KX]
            for piece in range(6):
                k0, k1 = wih_bounds[piece], wih_bounds[piece + 1]
                wih_engines[piece].dma_start(
                    wih_sb[:, k0:k1, :], wih_d[:, k0:k1, :]
                )

            whh_sb = constp.tile([128, NC, 3, G4], BF16, tag="whh")
            # zero rows cancel the junk rows of blob slot 2 (ops starting at
            # partition>0 are limited to 32 partitions -> 3 memsets)
            for p0 in (32, 64, 96):
                nc.vector.memset(whh_sb[p0 : p0 + 32, :, 2, :], 0.0)
            whh_engines = [nc.gpsimd, nc.sync, nc.scalar, nc.gpsimd]
            for piece in range(4):
                r0 = piece * 2
                whh_engines[piece].dma_start(
                    whh_sb[:, r0 : r0 + 2, 0:2, :], whhF_d[:, r0 : r0 + 2, :, :]
                )
            nc.sync.dma_start(whh_sb[0:32, :, 2, :], whhQ_d[:, :, :])

            xp_sb = constp.tile([16, NCH, G4], BF16, tag="xp")

            def xproj_chunk(c):
                px = pgp.tile([128, S], F32, tag="pg", name=f"px{c}")
                for k in range(KX):
                    for gi in range(4):
                        nc.tensor.matmul(
                            px[32 * gi : 32 * gi + 16, :],
                            lhsT=xT_sb[:, k, 16 * c : 16 * c + 16],
                            rhs=wih_sb[:, k, S * gi : S * (gi + 1)],
                            start=(k == 0),
                            stop=(k == KX - 1),
                            tile_position=(0, 32 * gi),
                        )
                for gi in range(4):
                    nc.vector.tensor_copy(
                        xp_sb[0:16, c, S * gi : S * (gi + 1)],
                        px[32 * gi : 32 * gi + 16, :],
                    )

            xproj_chunk(0)
            xproj_chunk(1)

            def inject(pg, s):
                cs = s if s < 8 else 15 - s
                eye_sl = eyes_sb[:, 0:16] if s < 8 else eyes_sb[:, 16:32]
                for gi in range(4):
                    nc.tensor.matmul(
                        pg[32 * gi : 32 * gi + 16, :],
                        lhsT=eye_sl,
                        rhs=xp_sb[0:16, cs, S * gi : S * (gi + 1)],
                        start=True,
                        stop=(s == 0),
                        tile_position=(0, 32 * gi),
                    )

            pg = pgp.tile([128, S], F32, tag="pg", name="pg0")
            inject(pg, 0)

            hTg_prev = None
            c_prev = None
            h15 = None
            for s in range(T):
                if s > 0:
                    for grp in range(NG):
                        r, c = divmod(grp, 3)
                        for gi in range(4):
                            nc.tensor.matmul(
                                pg[32 * gi : 32 * gi + 16, :],
                                lhsT=hTg_prev[:, r, 16 * c : 16 * c + 16],
                                rhs=whh_sb[:, r, c, S * gi : S * (gi + 1)],
                                start=False,
                                stop=(grp == NG - 1),
                                tile_position=(0, 32 * gi),
                            )

                # ---- epilogue: PSUM col groups i@0, f@32, o@64, g@96.
                # Cross-base ACTs realign every gate to partition base 0
                # (walrus only restricts InstTensorTensor to same-base APs).
                if s > 0:
                    sgf = ewp.tile([16, S], F32, tag="sgf")
                    nc.scalar.activation(sgf[:, :], pg[32:48, :], SIG)
                tg = ewp.tile([16, S], F32, tag="tg")
                nc.scalar.activation(tg[:, :], pg[96:112, :], TANH)
                sgi = ewp.tile([16, S], F32, tag="sgi")
                nc.scalar.activation(sgi[:, :], pg[0:16, :], SIG)
                sgo = ewp.tile([16, S], F32, tag="sgo")
                nc.scalar.activation(sgo[:, :], pg[64:80, :], SIG)
                if s > 0:
                    fc = ewp.tile([16, S], F32, tag="fc")
                    nc.vector.tensor_tensor(fc[:, :], sgf[:, :], c_prev[:, :], MULT)
                m1 = ewp.tile([16, S], F32, tag="m1")
                nc.vector.tensor_tensor(m1[:, :], sgi[:, :], tg[:, :], MULT)
                if s == 0:
                    c_new = m1
                else:
                    c_new = statep.tile([16, S], F32, tag="c")
                c_prev = c_new
                tc_ = ewp.tile([16, S], F32, tag="tc")

                CHUNKS = ((0, 0, 128), (1, 128, 256), (2, 256, S))
                if s < T - 1:
                    # two short dummies keep the PE HAM warm through the
                    # scalar/DVE epilogue chain without delaying the
                    # transposes queued behind them
                    dum = pdp.tile([16, S], F32, tag="dum")
                    for _ in range(2):
                        nc.tensor.matmul(
                            dum[:, 0:144], lhsT=tg[:, 0:16], rhs=tg[:, 0:144],
                            start=True, stop=True,
                        )

                    h = statep.tile([16, S], BF16, tag="h")
                    blob = blobp.tile([128, 48], BF16, tag="blob")
                    # slot-2 rows 32.. must be finite: stationary junk is
                    # multiplied by whh zero rows, but NaN*0 would poison PSUM
                    for p0 in (32, 64, 96):
                        nc.gpsimd.memset(blob[p0 : p0 + 32, 32:48], 0.0)
                    # column-chunked tail: c -> tanh(c) -> h -> transpose ->
                    # blob, pipelined across gpsimd/scalar/vector/PE
                    last_tr = None
                    for j, c0, c1 in CHUNKS:
                        if s == 0:
                            nc.scalar.activation(
                                tc_[:, c0:c1], m1[:, c0:c1], TANH
                            )
                        else:
                            nc.gpsimd.tensor_tensor(
                                c_new[:, c0:c1], fc[:, c0:c1], m1[:, c0:c1], ADD
                            )
                            nc.scalar.activation(
                                tc_[:, c0:c1], c_new[:, c0:c1], TANH
                            )
                        nc.vector.tensor_tensor(
                            h[:, c0:c1], sgo[:, c0:c1], tc_[:, c0:c1], MULT
                        )
                        tp = ptp.tile([128, 16], BF16, tag="tp")
                        wdt = c1 - c0
                        tr = nc.tensor.transpose(
                            tp[0:wdt, :], h[:, c0:c1], eyes_sb[:, 0:16]
                        )
                        last_tr = tr
                        nc.vector.tensor_copy(
                            blob[0:wdt, 16 * j : 16 * j + 16], tp[0:wdt, :]
                        )
                    # transport: blob -> DRAM -> AllGather -> hTg SBUF
                    cc_in = dp.tile([128, 48], BF16, tag="ccin")
                    nc.sync.dma_start(cc_in[:, :], blob[:, :])
                    cc_out = dp.tile(
                        [NC, 128, 48], BF16, addr_space="Shared", tag="ccout"
                    )
                    nc.gpsimd.collective_compute(
                        "AllGather",
                        mybir.AluOpType.bypass,
                        ins=[cc_in.opt()],
                        outs=[cc_out.opt()],
                        replica_groups=[list(range(NC))],
                    )
                    hTg_prev = hTgp.tile([128, NC, 48], BF16, tag="hTg")
                    for rr, eng in ((0, nc.sync), (4, nc.scalar)):
                        eng.dma_start(
                            hTg_prev[:, rr : rr + 4, :],
                            cc_out[rr : rr + 4, :, :].rearrange("r p e -> p r e"),
                        )
                    # gap work: future xproj chunk + next step's inject
                    if s + 2 <= NCH - 1:
                        xproj_chunk(s + 2)
                    pg = pgp.tile([128, S], F32, tag="pg", name=f"pg{s + 1}")
                    inject(pg, s + 1)
                    # dummy f32 matmuls bridge the remaining AllGather gap so
                    # the PE HAM clock-gate stays at 8/8
                    ndum = 8 if s + 2 <= NCH - 1 else 12
                    first_dum2 = None
                    for _ in range(ndum):
                        dmm = nc.tensor.matmul(
                            dum[:, :], lhsT=tg[:, 0:16], rhs=tg[:, 0:S],
                            start=True, stop=True,
                        )
                        if first_dum2 is None:
                            first_dum2 = dmm
                    add_dep_helper(
                        first_dum2.ins, last_tr.ins, False,
                        reason="dummies after transpose",
                    )
                else:
                    if s > 0:
                        nc.gpsimd.tensor_tensor(
                            c_new[:, :], fc[:, :], m1[:, :], ADD
                        )
                    nc.scalar.activation(tc_[:, :], c_new[:, :], TANH)
                    h15 = ewp.tile([16, S], F32, tag="h15")
                    nc.vector.tensor_tensor(h15[:, :], sgo[:, :], tc_[:, :], MULT)

            # ---- output: y[b] = leaky(w0*tanh(h_fwd[b]) + w1*tanh(h_bwd[b]) + cb)
            th = ewp.tile([16, S], F32, tag="th")
            nc.scalar.activation(th[:, :], h15[:, :], TANH)
            thb = ewp.tile([B, S], F32, tag="thb")
            nc.sync.dma_start(thb[:, :], th[8:16, :])
            yb = ewp.tile([B, S], F32, tag="yb")
            nc.vector.tensor_scalar(yb[:, :], thb[:, :], w1, cb, MULT, ADD)
            yc = ewp.tile([B, S], F32, tag="yc")
            nc.vector.scalar_tensor_tensor(yc[:, :], th[0:8, :], w0, yb[:, :], MULT, ADD)
            ye = ewp.tile([B, S], F32, tag="ye")
            nc.vector.scalar_tensor_tensor(ye[:, :], yc[:, :], 0.01, yc[:, :], MULT, MAX)
            nc.sync.dma_start(out_d[:, :], ye[:, :])

    nc.compile()
    return nc


def _prep_inputs(x, W_ih, W_hh, b_ih, b_hh):
    """Build the 8 per-core input maps (SBUF-layout pre-permuted)."""
    X = np.ascontiguousarray(x, dtype=np.float32).reshape(B, T, HW)
    # sequence-row layout (matches the reference's stack+reshape pairing):
    # rows 0-3 fwd samples (0,2,4,6); 4-7 bwd (0,2,4,6);
    # rows 8-11 fwd (1,3,5,7); 12-15 bwd (1,3,5,7)
    SAMP = [0, 2, 4, 6, 0, 2, 4, 6, 1, 3, 5, 7, 1, 3, 5, 7]
    ISFWD = [1, 1, 1, 1, 0, 0, 0, 0, 1, 1, 1, 1, 0, 0, 0, 0]
    cols = np.zeros((HW, 128), dtype=np.float32)
    for ch in range(NCH):
        for j in range(16):
            t = ch if ISFWD[j] else T - 1 - ch
            cols[:, 16 * ch + j] = X[SAMP[j], t, :]
    xTfull = np.zeros((KX * 128, 128), dtype=np.float32)
    xTfull[:HW] = cols
    xTfull[HW] = 1.0  # bias ones-row (k-tile 18, partition 0)
    xT = np.ascontiguousarray(
        xTfull.reshape(KX, 128, 128).transpose(1, 0, 2)
    ).astype(ml_dtypes.bfloat16)

    bias = (b_ih + b_hh).astype(np.float32)
    eye = np.eye(16, dtype=np.float32)
    # steps s>=8 reuse chunk 15-s with fwd/bwd row groups swapped (j ^ 4)
    eyeS = np.zeros((16, 16), dtype=np.float32)
    eyeS[np.arange(16) ^ 4, np.arange(16)] = 1.0
    eyes = np.concatenate([eye, eyeS], axis=1).astype(ml_dtypes.bfloat16)

    in_maps = []
    for q in range(NC):
        units = np.arange(S * q, S * (q + 1))
        # col-group order i, f, o, g  (torch gate blocks i=0, f=1, g=2, o=3)
        rows = np.concatenate([g * HW + units for g in (0, 1, 3, 2)])
        Wi = W_ih[rows].T.astype(np.float32)  # [HW, G4]
        wih_full = np.zeros((KX * 128, G4), dtype=np.float32)
        wih_full[:HW] = Wi
        wih_full[HW] = bias[rows]
        wih = np.ascontiguousarray(
            wih_full.reshape(KX, 128, G4).transpose(1, 0, 2)
        ).astype(ml_dtypes.bfloat16)

        Wt = W_hh[rows].T.astype(np.float32)  # [HW(contraction), G4]
        whhF = np.zeros((128, NC, 2, G4), dtype=np.float32)
        whhQ = np.zeros((32, NC, G4), dtype=np.float32)
        for r in range(NC):
            for c in range(2):
                whhF[:, r, c, :] = Wt[S * r + 128 * c : S * r + 128 * c + 128, :]
            whhQ[:, r, :] = Wt[S * r + 256 : S * r + 288, :]
        in_maps.append(
            {
                "xT": xT,
                "wih": wih,
                "whhF": whhF.astype(ml_dtypes.bfloat16),
                "whhQ": whhQ.astype(ml_dtypes.bfloat16),
                "eyes": eyes,
            }
        )
    return in_maps


def run(x, W_ih, W_hh, b_ih, b_hh, conv_w, conv_b, trace=False, tmpdir=None):
    """Build + run on 8 cores; returns (full_output, BassKernelResults)."""
    w0 = float(np.asarray(conv_w).reshape(2)[0])
    w1 = float(np.asarray(conv_w).reshape(2)[1])
    cb = float(np.asarray(conv_b).reshape(1)[0])
    nc = _build(w0, w1, cb)
    in_maps = _prep_inputs(
        np.asarray(x), np.asarray(W_ih), np.asarray(W_hh),
        np.asarray(b_ih), np.asarray(b_hh),
    )
    res = bass_utils.run_bass_kernel_spmd(
        nc, in_maps, core_ids=list(range(NC)), trace=trace, tmpdir=tmpdir
    )
    y = np.empty((B, HW), dtype=np.float32)
    for q in range(NC):
        y[:, S * q : S * (q + 1)] = res.results[q]["out"]
    return y.reshape(B, 1, H, W).astype(np.float32), res


def kernel(x, W_ih, W_hh, b_ih, b_hh, conv_w, conv_b):
    y, _ = run(x, W_ih, W_hh, b_ih, b_hh, conv_w, conv_b, trace=False)
    return y
